# revision 1
# baseline (speedup 1.0000x reference)
"""Trainium2 Bass kernel v2 for spatial self-attention (nn_Attention_90615220011343).

Per-core math (core c -> batch c//2, heads 2*(c%2), 2*(c%2)+1):
    qkv = x @ w_qkv; per head sim^T[j,i] = k^T q; attn = softmax; out = attn@v
    y_partial = sum_h (out_h/den) @ wo_h ; host sums head-pairs + bias.

Key cost-model facts exploited (CoreSim instruction_cost_v2):
  - matmul engine time = out_free_size * cycles_per_row only (K, M free;
    Ldweights is free). fp32r = 1.0 c/r when free >= 256, bf16 = 1.0 always,
    fp32r = 4.0 when free < 256.
  - attn@v computed as out[i=128part, 33free] with K=j=128 (lhsT = exp slab
    block, rhs = v_aug[j,33] in bf16) -> 33 cycles per matmul instead of 512.
    Ones column of v_aug makes column 32 the softmax denominator, already in
    [i-partition, 1] layout (no transposes for the denominator).
  - exp work split ACT (cycle 0.833ns/el) + Pool gpsimd (1.389ns/el):
    24-chunk rhythm [A4 P2 A4 P2 A4 P2 A3 P3] = 15:9 chunk split.
  - PSUM: one rotating 6-bank region for sim chunks (512 cols each), 1 bank
    for av accumulation (132 cols), 1 bank for transposes + y projections.
"""

import numpy as np

HEADS = 4
DH = 32
N = 4096
C = 256
P = 128
NCH = 32          # j-chunks of 128 tokens
ITILES = 8        # i tiles of 512
ROT = 6           # rotating psum banks for sim chunks
EROT = 12         # eslab rotation depth (chunks)
AV_LAG = 10       # chunks between sim emission and its av matmuls
# exp quanta pattern (engine, nchunks): "A" = exact exp on ACT,
# "V" = Schraudolph bf16 exp on DVE (bit-trick: round(s*a+b) as int16 IS
# bf16(exp(s)) up to a +-4% sawtooth; softmax normalization cancels most).
EXP_PATTERN = [("V", 2) if (i * 12) // 32 != ((i + 1) * 12) // 32 else ("A", 2)
               for i in range(32)]
SCH_A = float(2 ** 7 / np.log(2))
SCH_B = float(127 * 2 ** 7) - 7.6

_CACHED = {}


def _build_nc():
    import concourse.bass as bass
    import concourse.mybir as mybir
    from concourse.tile import TileContext
    from concourse.masks import make_identity

    FP = mybir.dt.float32
    FR = mybir.dt.float32r
    BF = mybir.dt.bfloat16
    AF = mybir.ActivationFunctionType
    ALU = mybir.AluOpType

    import os
    debug = bool(os.environ.get("K2_DEBUG"))
    nc = bass.Bass(target_bir_lowering=False)
    U16 = mybir.dt.uint16
    xt_d = nc.declare_dram_parameter("xt", [C, N], U16, isOutput=False)
    if debug:
        dbg_outT = nc.declare_dram_parameter("dbg_outT", [64, N], FP, isOutput=True)
        dbg_rden = nc.declare_dram_parameter("dbg_rden", [P, 64], FP, isOutput=True)
        dbg_qrep = nc.declare_dram_parameter("dbg_qrep", [P, N], FP, isOutput=True)
        dbg_karr = nc.declare_dram_parameter("dbg_karr", [P, N // 4], FP, isOutput=True)
        dbg_vaug = nc.declare_dram_parameter("dbg_vaug", [P, 33 * NCH], FP, isOutput=True)
        dbg_yacc = nc.declare_dram_parameter("dbg_yacc", [P, NCH * C], FP, isOutput=True)
    wq_d = nc.declare_dram_parameter("wq", [C, 2 * P], FP, isOutput=False)
    wk_d = nc.declare_dram_parameter("wk", [C, 64], FP, isOutput=False)
    wv_d = nc.declare_dram_parameter("wv", [C, 64], FP, isOutput=False)
    wo_d = nc.declare_dram_parameter("wo", [64, C], FP, isOutput=False)
    y_d = nc.declare_dram_parameter("y", [N, C], FP, isOutput=True)

    with TileContext(nc) as tc:
        with (
            tc.tile_pool(name="const", bufs=1) as constp,
            tc.tile_pool(name="big", bufs=1) as bigp,
            tc.tile_pool(name="ytmp", bufs=4) as ytmpp,
            tc.tile_pool(name="psR", bufs=1, space="PSUM") as psR,
            tc.tile_pool(name="psV", bufs=1, space="PSUM") as psV,
            tc.tile_pool(name="psT", bufs=1, space="PSUM") as psT,
        ):
            ident = constp.tile([P, P], FP, tag="ident")
            make_identity(nc, ident[:])
            identr = constp.tile([P, P], FR, tag="identr")
            nc.vector.tensor_copy(out=identr[:], in_=ident[:])

            # ---- persistent SBUF ----
            xT = [bigp.tile([P, N], BF, tag=f"xT{cc}", name=f"xT{cc}") for cc in range(2)]
            qrep = [bigp.tile([P, N], FR, tag=f"qrep{h}", name=f"qrep{h}") for h in range(2)]
            karr = [bigp.tile([P, N // 4], FR, tag=f"karr{h}", name=f"karr{h}") for h in range(2)]
            vaug = [bigp.tile([P, 33 * NCH], BF, tag=f"vaug{h}", name=f"vaug{h}") for h in range(2)]
            outT = [bigp.tile([32, N], FR, tag=f"outT{h}", name=f"outT{h}")
                    for h in range(2)]
            rden = bigp.tile([P, 64], FP, tag="rden")
            eslabs = [bigp.tile([P, 1024], BF, tag=f"esl{t}", name=f"esl{t}")
                      for t in range(EROT // 2)]
            av_sc = bigp.tile([P, P], FR, tag="av_sc")

            wq_sb = bigp.tile([P, 2, 2 * P], BF, tag="wq")
            wk_sb = bigp.tile([P, 2, 64], BF, tag="wk")
            wv_sb = bigp.tile([P, 2, 64], BF, tag="wv")
            wo_sb = [bigp.tile([32, C], FR, tag=f"wo{h}", name=f"wo{h}")
                     for h in range(2)]

            # ---- psum ----
            # 3 tiles x 2 banks: separate tile objects keep Tile's
            # (tile-granular) dependency tracking precise per 2-bank slot.
            rots = [psR.tile([P, 1024], FP, tag=f"R{t}", name=f"rotT{t}")
                    for t in range(ROT // 2)]
            avp = psV.tile([P, 512], FP, tag="V")      # use cols 0:132
            tb = psT.tile([P, 512], FP, tag="T")

            def rhalf(bc):
                # half-bank-pair slot for a rotating cursor value
                return rots[(bc % ROT) // 2], 512 * (bc % 2)

            # ---- weight loads + conversion ----
            wq_st = bigp.tile([P, 2, 2 * P], FP, tag="wq_st")
            wk_st = bigp.tile([P, 2, 64], FP, tag="wk_st")
            wv_st = bigp.tile([P, 2, 64], FP, tag="wv_st")
            wo_st = bigp.tile([64, C], FP, tag="wo_st")
            for cc in range(2):
                nc.sync.dma_start(out=wq_st[:, cc, :], in_=wq_d[cc * P:(cc + 1) * P, :])
                nc.sync.dma_start(out=wk_st[:, cc, :], in_=wk_d[cc * P:(cc + 1) * P, :])
                nc.sync.dma_start(out=wv_st[:, cc, :], in_=wv_d[cc * P:(cc + 1) * P, :])
            nc.sync.dma_start(out=wo_st[:], in_=wo_d[:])
            nc.vector.tensor_copy(out=wq_sb[:], in_=wq_st[:])
            nc.vector.tensor_copy(out=wk_sb[:], in_=wk_st[:])
            nc.vector.tensor_copy(out=wv_sb[:], in_=wv_st[:])
            nc.vector.tensor_copy(out=wo_sb[0][:], in_=wo_st[0:32, :])
            nc.vector.tensor_copy(out=wo_sb[1][:], in_=wo_st[32:64, :])

            # ================= prologue: xT load (pre-transposed bf16 from
            # host), then v/qrep/karr builds straight from SBUF ============
            dma_engines = [nc.scalar, nc.sync]
            bankc = 0  # global rotating-slot cursor

            def qrep_chunk(h, it):
                nonlocal bankc
                rt, c0 = rhalf(bankc)
                bankc += 1
                for cc in range(2):
                    nc.tensor.matmul(
                        rt[:, c0:c0 + 512],
                        lhsT=wq_sb[:, cc, P * h:P * (h + 1)],
                        rhs=xT[cc][:, 512 * it:512 * (it + 1)],
                        start=(cc == 0), stop=(cc == 1),
                    )
                nc.vector.tensor_copy(
                    out=qrep[h][:, 512 * it:512 * (it + 1)],
                    in_=rt[:, c0:c0 + 512],
                )

            def karr_chunk(h, p_, ct):
                nonlocal bankc
                rt, c0 = rhalf(bankc)
                bankc += 1
                for cc in range(2):
                    xv = xT[cc][:].rearrange(
                        "q (m t f) -> q m t f", t=4, f=P
                    )[:, 4 * p_:4 * p_ + 4, ct, :]
                    nc.tensor.matmul(
                        rt[0:32, c0:c0 + 512],
                        lhsT=wk_sb[:, cc, 32 * h:32 * (h + 1)],
                        rhs=xv,
                        start=(cc == 0), stop=(cc == 1),
                    )
                nc.vector.tensor_copy(
                    out=karr[h][32 * ct:32 * (ct + 1),
                                512 * p_:512 * (p_ + 1)],
                    in_=rt[0:32, c0:c0 + 512],
                )

            for cc in range(2):
                for s in range(4):
                    dma_engines[s % 2].dma_start(
                        out=xT[cc][:, 1024 * s:1024 * (s + 1)].bitcast(U16),
                        in_=xt_d[P * cc:P * (cc + 1),
                                 1024 * s:1024 * (s + 1)],
                    )

            ones_st = bigp.tile([P, NCH], BF, tag="ones_st")
            nc.gpsimd.memset(ones_st[:], 1.0)
            for h in range(2):
                vv = vaug[h][:].rearrange("p (k e) -> p k e", e=33)
                nc.vector.tensor_copy(out=vv[:, :, 32], in_=ones_st[:])

            def v_round(k0):
                nonlocal bankc
                rt2, c02 = rhalf(bankc)
                bankc += 1
                for k in range(k0, k0 + 4):
                    for cc in range(2):
                        nc.tensor.matmul(
                            rt2[:, c02 + 64 * (k - k0):
                                c02 + 64 * (k - k0) + 64],
                            lhsT=xT[cc][:, P * k:P * (k + 1)],
                            rhs=wv_sb[:, cc, :],
                            start=(cc == 0), stop=(cc == 1),
                        )
                sv2 = rt2[:, c02: c02 + 256].rearrange("p (k d) -> p k d", d=64)
                for h in range(2):
                    vv = vaug[h][:].rearrange("p (k e) -> p k e", e=33)
                    nc.vector.tensor_copy(
                        out=vv[:, k0:k0 + 4, 0:32],
                        in_=sv2[:, :, 32 * h:32 * (h + 1)],
                    )

            for r in range(4):
                qrep_chunk(0, 2 * r)
                qrep_chunk(0, 2 * r + 1)
                v_round(8 * r)
                v_round(8 * r + 4)
            for p_ in range(2):
                for ct in range(4):
                    karr_chunk(0, p_, ct)

            # ================= attention chunk stream ======================
            # global chunk c -> (h, it, j); sim -> rot bank c%ROT; exp quanta
            # per EXP_PATTERN; av lags AV_LAG chunks; per-i-tile epilogue
            # (stage/recip/transpose/copyT) hooks; y projections of i-tile
            # t-1 of the OTHER-completed head run interleaved.
            def chunk_meta(c):
                h = c // (ITILES * NCH)
                it = (c // NCH) % ITILES
                j = c % NCH
                return h, it, j

            # exp quantum boundaries (start chunk -> (engine, len));
            # generated per-head so no quantum spans the head boundary
            # (the interhead qkv build reuses ROT banks).
            quanta = {}
            HB = ITILES * NCH
            import itertools
            for h0 in (0, HB):
                cpos = 0
                pat = itertools.cycle(EXP_PATTERN)
                while cpos < HB:
                    eng, ln = next(pat)
                    quanta[h0 + cpos] = (eng, ln)
                    cpos += ln

            NC_TOT = 2 * ITILES * NCH

            slot_of = {}

            def emit_sim(c):
                nonlocal bankc
                h, it, j = chunk_meta(c)
                slot_of[c] = bankc
                rt_, c0 = rhalf(bankc)
                bankc += 1
                rp = j % 4
                nc.tensor.matmul(
                    rt_[:, c0:c0 + 512],
                    lhsT=karr[h][32 * rp:32 * (rp + 1),
                                 P * (j // 4):P * (j // 4 + 1)],
                    rhs=qrep[h][32 * rp:32 * (rp + 1),
                                512 * it:512 * (it + 1)],
                    start=True, stop=True,
                    tile_position=(32 * rp, 0),
                )

            I16 = mybir.dt.int16

            def _exp_one(es_ap, rt_ap, eng):
                if eng == "A":
                    nc.scalar.activation(es_ap, rt_ap, AF.Exp)
                else:
                    nc.vector.tensor_scalar(
                        out=es_ap.bitcast(I16), in0=rt_ap,
                        scalar1=SCH_A, scalar2=SCH_B,
                        op0=ALU.mult, op1=ALU.add,
                    )

            def emit_exp(c0, eng, ln):
                assert c0 % 2 == 0 and ln == 2, (c0, ln)
                s0 = slot_of[c0]
                assert s0 % 2 == 0 and slot_of[c0 + 1] == s0 + 1, (c0, s0)
                rt_ = rots[(s0 % ROT) // 2]
                es = eslabs[(c0 % EROT) // 2]
                _exp_one(es[:], rt_[:], eng)

            def emit_av(c):
                h, it, j = chunk_meta(c)
                es = eslabs[(c % EROT) // 2]
                e0 = 512 * (c % 2)
                for ic in range(4):
                    nc.tensor.matmul(
                        avp[:, 33 * ic:33 * ic + 33],
                        lhsT=es[:, e0 + 128 * ic:e0 + 128 * (ic + 1)],
                        rhs=vaug[h][:, 33 * j:33 * j + 33],
                        start=(j == 0 and ic == 0), stop=(j == NCH - 1),
                        skip_group_check=True,
                    )

            def emit_itile_stage(h, it):
                # reciprocal of dens from psum, then 4 scaled stages
                # (avp out-cols * 1/den -> av_sc); scaling here (per-partition
                # = per-i) lets both heads' y projections share one psum
                # accumulation later.
                dv = avp[:, 0:132].rearrange("p (ic e) -> p ic e", e=33)[:, :, 32]
                r0 = 32 * h + 4 * it
                nc.vector.reciprocal(out=rden[:, r0:r0 + 4], in_=dv)
                for ic in range(4):
                    nc.vector.tensor_scalar_mul(
                        av_sc[:, 32 * ic:32 * (ic + 1)],
                        avp[:, 33 * ic:33 * ic + 32],
                        rden[:, r0 + ic:r0 + ic + 1],
                    )

            def emit_itile_transpose(h, it):
                for ic in range(4):
                    nc.tensor.transpose(
                        tb[0:32, 128 * ic:128 * (ic + 1)].bitcast(FR),
                        av_sc[:, 32 * ic:32 * (ic + 1)],
                        identr[:],
                    )
                nc.vector.tensor_copy(
                    out=outT[h][:, 512 * it:512 * (it + 1)],
                    in_=tb[0:32, 0:512].bitcast(FR),
                )

            def emit_y(it):
                # both heads' projections of chunk k accumulate in one psum
                # region (outT rows already den-normalized), then store.
                for ic in range(4):
                    k = 4 * it + ic
                    cols = slice(256 * (ic % 2), 256 * (ic % 2) + C)
                    for h in range(2):
                        nc.tensor.matmul(
                            tb[:, cols],
                            lhsT=outT[h][:, P * k:P * (k + 1)],
                            rhs=wo_sb[h][:],
                            start=(h == 0), stop=(h == 1),
                            tile_position=(0, 0),
                        )
                    yo = ytmpp.tile([P, C], FP, tag="yo")
                    nc.vector.tensor_copy(out=yo[:], in_=tb[:, cols])
                    nc.sync.dma_start(out=y_d[P * k:P * (k + 1), :], in_=yo[:])

            # pending per-chunk hook queues keyed by emission chunk index
            hooks = {}

            def add_hook(c, fn):
                hooks.setdefault(min(c, NC_TOT - 1), []).append(fn)

            # head-1 qkv injected into att(0) as tile-aligned 2-slot units
            qkv_units = ([("q", it) for it in range(0, ITILES, 2)]
                         + [("k", p_, ct) for p_ in range(2)
                            for ct in range(0, 4, 2)])
            inject_at = {32: 1, 64: 1, 96: 1, 128: 1, 160: 1, 192: 1, 224: 2}

            def emit_qkv_unit(u):
                if u[0] == "q":
                    qrep_chunk(1, u[1])
                    qrep_chunk(1, u[1] + 1)
                else:
                    karr_chunk(1, u[1], u[2])
                    karr_chunk(1, u[1], u[2] + 1)

            for c in range(NC_TOT):
                h, it, j = chunk_meta(c)
                for _ in range(inject_at.get(c, 0)):
                    emit_qkv_unit(qkv_units.pop(0))
                emit_sim(c)
                if c >= AV_LAG:
                    # av of c-AV_LAG MUST precede the exp quantum closing at c:
                    # that exp overwrites the eslab cols av(c-AV_LAG) reads.
                    emit_av(c - AV_LAG)
                    ch, cit, cj = chunk_meta(c - AV_LAG)
                    if cj == NCH - 1:
                        emit_itile_stage(ch, cit)
                        add_hook(c + 2, lambda ch=ch, cit=cit:
                                 emit_itile_transpose(ch, cit))
                        if ch == 1 and cit > 0:
                            add_hook(c + 4, lambda cit=cit: emit_y(cit - 1))
                if (c + 1) in quanta or c + 1 == NC_TOT:
                    # close the quantum that ENDS at chunk c
                    q0 = max(q for q in quanta if q <= c)
                    eng, ln = quanta[q0]
                    emit_exp(q0, eng, min(ln, NC_TOT - q0))
                for fn in hooks.pop(c, ()):
                    fn()

            # tail: remaining avs, last i-tile stage/transpose, last y projs
            for c in range(NC_TOT - AV_LAG, NC_TOT):
                emit_av(c)
                ch, cit, cj = chunk_meta(c)
                if cj == NCH - 1:
                    emit_itile_stage(ch, cit)
                    emit_itile_transpose(ch, cit)
            for fn_list in [hooks[k] for k in sorted(hooks)]:
                for fn in fn_list:
                    fn()
            emit_y(ITILES - 2)
            emit_y(ITILES - 1)

            if debug:
                dbt = bigp.tile([P, N], FP, tag="dbt")
                nc.vector.tensor_copy(out=dbt[0:32, 0:N], in_=outT[0][:])
                nc.sync.dma_start(out=dbg_outT[:], in_=dbt[0:64, 0:N])
                nc.sync.dma_start(out=dbg_rden[:], in_=rden[:])
                nc.vector.tensor_copy(out=dbt[:, 0:N], in_=qrep[0][:])
                nc.sync.dma_start(out=dbg_qrep[:], in_=dbt[:, 0:N])
                nc.vector.tensor_copy(out=dbt[:, 0:N // 4], in_=karr[0][:])
                nc.sync.dma_start(out=dbg_karr[:], in_=dbt[:, 0:N // 4])
                nc.vector.tensor_copy(out=dbt[:, 0:33 * NCH], in_=vaug[0][:])
                nc.sync.dma_start(out=dbg_vaug[:], in_=dbt[:, 0:33 * NCH])


    _split_excess_waits(nc, mybir)
    return nc


def _split_excess_waits(nc, mybir, maxw=1, carrier_cap=1):
    """Hoist excess semaphore waits onto InstEventSemaphore carriers."""
    skip = {
        "InstEventSemaphore", "InstCall",
        "InstUnconditionalBranch", "InstISA", "InstRegisterMove",
    }
    for f in nc.m.functions:
        for blk in f.blocks:
            idx = 0
            while idx < len(blk.instructions):
                ins = blk.instructions[idx]
                si = getattr(ins, "sync_info", None)
                if (
                    si is not None and si.on_wait and len(si.on_wait) > maxw
                    and type(ins).__name__ not in skip
                ):
                    waits = list(si.on_wait)
                    keep, excess = waits[:maxw], waits[maxw:]
                    # keep Ldweights/Matmult pairs adjacent: walrus LDW
                    # optimization requires it, so hoist carriers above the
                    # Ldweights when one immediately precedes.
                    at = idx
                    if (at > 0 and type(blk.instructions[at - 1]).__name__
                            == "InstLdweights"):
                        at -= 1
                    n_ins = 0
                    for i in range(0, len(excess), carrier_cap):
                        ev = mybir.InstEventSemaphore(
                            name=nc.get_next_instruction_name(),
                            engine=ins.engine,
                            ins=[], outs=[],
                            sync_info=mybir.SyncInfo(
                                on_wait=excess[i:i + carrier_cap], on_update=[]
                            ),
                        )
                        nc.register_instruction(ev)
                        blk.instructions.insert(at + n_ins, ev)
                        n_ins += 1
                    ins.sync_info = mybir.SyncInfo(
                        on_wait=keep, on_update=list(si.on_update or [])
                    )
                    idx += n_ins
                idx += 1
    return nc


def get_nc():
    if "nc" not in _CACHED:
        _CACHED["nc"] = _build_nc()
    return _CACHED["nc"]


def make_in_maps(x, w_qkv, w_out):
    """Host-side sharding: core c -> batch c//2, heads (c%2)*2, (c%2)*2+1."""
    B = x.shape[0]
    xf = np.ascontiguousarray(x.reshape(B, N, C))
    scale = DH ** -0.5
    in_maps = []
    for core in range(8):
        b, hp = core // 2, core % 2
        h0, h1 = 2 * hp, 2 * hp + 1
        wq = np.concatenate(
            [np.tile(w_qkv[:, h * DH:(h + 1) * DH] * scale, (1, 4)) for h in (h0, h1)],
            axis=1,
        )
        wk = np.concatenate(
            [w_qkv[:, 128 + h * DH: 128 + (h + 1) * DH] for h in (h0, h1)], axis=1
        )
        wv = np.concatenate(
            [w_qkv[:, 256 + h * DH: 256 + (h + 1) * DH] for h in (h0, h1)], axis=1
        )
        wo = np.concatenate(
            [w_out[h * DH:(h + 1) * DH, :] for h in (h0, h1)], axis=0
        )
        import ml_dtypes
        in_maps.append({
            "xt": np.ascontiguousarray(xf[b].T.astype(ml_dtypes.bfloat16)).view(np.uint16),
            "wq": np.ascontiguousarray(wq.astype(np.float32)),
            "wk": np.ascontiguousarray(wk.astype(np.float32)),
            "wv": np.ascontiguousarray(wv.astype(np.float32)),
            "wo": np.ascontiguousarray(wo.astype(np.float32)),
        })
    return in_maps


def kernel(x, w_qkv, w_out, b_out):
    from concourse.bass_utils import run_bass_kernel_spmd

    nc = get_nc()
    in_maps = make_in_maps(
        np.asarray(x, dtype=np.float32),
        np.asarray(w_qkv, dtype=np.float32),
        np.asarray(w_out, dtype=np.float32),
    )
    res = run_bass_kernel_spmd(nc, in_maps, list(range(8))).results
    B, H, W = 4, 64, 64
    y = np.empty((B, N, C), dtype=np.float32)
    for b in range(B):
        y[b] = res[2 * b]["y"] + res[2 * b + 1]["y"]
    y += np.asarray(b_out, dtype=np.float32)
    return y.reshape(B, H, W, C)



# revision 4
# speedup vs baseline: 1.0401x; 1.0401x over previous
"""Trainium2 Bass kernel v3 for spatial self-attention (nn_Attention_90615220011343).

Per-core (core c -> batch c//2, heads 2*(c%2), 2*(c%2)+1):
    qkv = x @ w_qkv; per head sim^T[j,i] = k^T q; attn = softmax; out = attn@v
    y_partial = sum_h (out_h/den) @ wo_h ; host sums head-pairs + bias.

v3 changes vs v2 (cost-model driven):
  - sim matmul in fp8e4m3 with MatmulPerfMode.DoubleRow: 0.5 cycles/row
    (vs 1.0 for bf16/fp32r) -> 256 PE cycles per [128,512] chunk.
    Precision recovered by error compensation: q = qhi + qlo, k = khi + klo
    (hi = fp8(x), lo = fp8(x - hi)); the 128 DoubleRow contraction slots
    (64 partitions x 2) hold all four cross products (qhi+qlo)x(khi+klo),
    so the product is exact up to the ~0.1% lo-rounding. attn scale (1/sqrt(32))
    is folded into the exp input scaling, not into q.
  - exp runs on THREE engines: ACT (exact, activation Exp with scale),
    DVE + Pool (Schraudolph int16 bitcast = bf16 exp approx). Pattern
    weighted by engine rates (ACT 0.83, DVE 1.04, Pool 1.39 ns/row).
  - q^T/k^T builds write 2-itile/panel stacks into one [128,512] psum
    ([64*s + 32*h + d] partitions), so the fp8 hi-copy + lo-subtract are
    [128,512] ops (4x fewer engine rows); SBUF->SBUF DMAs (cheap issue from
    the gpsimd ring) fold the stacks into the DoubleRow operand layouts:
      qSide[h]: [64, 2, N] rows = (qhi d | qlo d), t duplicated
      kSide[h]: [64, 2, N] cols t = (khi | klo), rows duplicated
  - transposes of the normalized attention output are bf16 (1 c/r, vs fp32r
    4 c/r when free<256); outT copy reads bf16 psum (DVE 2x_1p mode).
"""

import numpy as np

HEADS = 4
DH = 32
N = 4096
C = 256
P = 128
NCH = 32          # j-chunks of 128 tokens
ITILES = 8        # i tiles of 512
ROT = 6           # rotating psum banks for sim chunks
EROT = 12         # eslab rotation depth (chunks)
AV_LAG = 10       # chunks between sim emission and its av matmuls
SCALE = float(DH ** -0.5)
# bf16 Schraudolph exp: round(s*a+b) as int16 IS bf16(exp(s)) up to ~3%
# sawtooth; softmax normalization cancels most. a absorbs the attn scale.
SCH_A = float(2 ** 7 / np.log(2)) * SCALE
SCH_B = float(127 * 2 ** 7) - 7.6
# exp engine pattern per 32 quanta (1 quantum = 2 chunks = [128,1024]):
# weighted by engine throughput ACT:DVE:Pool ~ 13:11:8
EXP_W = {"A": 13, "V": 11, "P": 8}

_CACHED = {}


def _make_pattern(total, weights):
    acc = {k: 0.0 for k in weights}
    wsum = float(sum(weights.values()))
    out = []
    for _ in range(total):
        for k in acc:
            acc[k] += weights[k]
        kbest = max(acc, key=lambda kk: (acc[kk], kk))
        acc[kbest] -= wsum
        out.append(kbest)
    return out


def _build_nc():
    import concourse.bass as bass
    import concourse.mybir as mybir
    from concourse.tile import TileContext
    from concourse.masks import make_identity

    FP = mybir.dt.float32
    BF = mybir.dt.bfloat16
    E4 = mybir.dt.float8e4
    U16 = mybir.dt.uint16
    I16 = mybir.dt.int16
    AF = mybir.ActivationFunctionType
    ALU = mybir.AluOpType
    DR = mybir.MatmulPerfMode.DoubleRow

    nc = bass.Bass(target_bir_lowering=False)
    xt_d = nc.declare_dram_parameter("xt", [C, N], U16, isOutput=False)
    wq_d = nc.declare_dram_parameter("wq", [C, 64], FP, isOutput=False)
    wk_d = nc.declare_dram_parameter("wk", [C, 64], FP, isOutput=False)
    wv_d = nc.declare_dram_parameter("wv", [C, 64], FP, isOutput=False)
    wo_d = nc.declare_dram_parameter("wo", [64, C], FP, isOutput=False)
    y_d = nc.declare_dram_parameter("y", [N, C], FP, isOutput=True)

    with TileContext(nc) as tc:
        with (
            tc.tile_pool(name="const", bufs=1) as constp,
            tc.tile_pool(name="big", bufs=1) as bigp,
            tc.tile_pool(name="stage", bufs=4) as stagep,
            tc.tile_pool(name="ytmp", bufs=4) as ytmpp,
            tc.tile_pool(name="psR", bufs=1, space="PSUM") as psR,
            tc.tile_pool(name="psV", bufs=1, space="PSUM") as psV,
            tc.tile_pool(name="psT", bufs=1, space="PSUM") as psT,
        ):
            ident = constp.tile([P, P], FP, tag="ident")
            make_identity(nc, ident[:])
            identb = constp.tile([P, P], BF, tag="identb")
            nc.vector.tensor_copy(out=identb[:], in_=ident[:])

            # ---- persistent SBUF ----
            xT = [bigp.tile([P, N], BF, tag=f"xT{cc}", name=f"xT{cc}") for cc in range(2)]
            qSide = [bigp.tile([64, 2, N], E4, tag=f"qS{h}", name=f"qS{h}")
                     for h in range(2)]
            kSide = [bigp.tile([64, 2, N], E4, tag=f"kS{h}", name=f"kS{h}")
                     for h in range(2)]
            vaug = [bigp.tile([P, 33 * NCH], BF, tag=f"vaug{h}", name=f"vaug{h}")
                    for h in range(2)]
            outT = [bigp.tile([32, N], BF, tag=f"outT{h}", name=f"outT{h}")
                    for h in range(2)]
            rden = bigp.tile([P, 64], FP, tag="rden")
            av_sc = bigp.tile([P, P], BF, tag="av_sc")
            eslabs = [bigp.tile([P, 1024], BF, tag=f"esl{t}", name=f"esl{t}")
                      for t in range(EROT // 2)]

            wq_sb = bigp.tile([P, 2, 64], BF, tag="wq")
            wk_sb = bigp.tile([P, 2, 64], BF, tag="wk")
            wv_sb = bigp.tile([P, 2, 64], BF, tag="wv")
            wo_sb = [bigp.tile([32, C], BF, tag=f"wo{h}", name=f"wo{h}")
                     for h in range(2)]

            # ---- psum ----
            rots = [psR.tile([P, 1024], FP, tag=f"R{t}", name=f"rotT{t}")
                    for t in range(ROT // 2)]
            avp = psV.tile([P, 512], FP, tag="V")      # cols 0:132 in use
            tb = psT.tile([P, 512], FP, tag="T")       # y projections

            def rhalf(bc):
                return rots[(bc % ROT) // 2], 512 * (bc % 2)

            # ---- weight loads + conversion ----
            wq_st = bigp.tile([P, 2, 64], FP, tag="wq_st")
            wk_st = bigp.tile([P, 2, 64], FP, tag="wk_st")
            wv_st = bigp.tile([P, 2, 64], FP, tag="wv_st")
            wo_st = bigp.tile([64, C], FP, tag="wo_st")
            for cc in range(2):
                nc.sync.dma_start(out=wq_st[:, cc, :], in_=wq_d[cc * P:(cc + 1) * P, :])
                nc.sync.dma_start(out=wk_st[:, cc, :], in_=wk_d[cc * P:(cc + 1) * P, :])
                nc.sync.dma_start(out=wv_st[:, cc, :], in_=wv_d[cc * P:(cc + 1) * P, :])
            nc.sync.dma_start(out=wo_st[:], in_=wo_d[:])
            nc.vector.tensor_copy(out=wq_sb[:], in_=wq_st[:])
            nc.vector.tensor_copy(out=wk_sb[:], in_=wk_st[:])
            nc.vector.tensor_copy(out=wv_sb[:], in_=wv_st[:])
            nc.vector.tensor_copy(out=wo_sb[0][:], in_=wo_st[0:32, :])
            nc.vector.tensor_copy(out=wo_sb[1][:], in_=wo_st[32:64, :])

            # ---- x load (pre-transposed bf16 from host) ----
            dma_engines = [nc.scalar, nc.sync]
            for cc in range(2):
                for s in range(4):
                    dma_engines[s % 2].dma_start(
                        out=xT[cc][:, 1024 * s:1024 * (s + 1)].bitcast(U16),
                        in_=xt_d[P * cc:P * (cc + 1),
                                 1024 * s:1024 * (s + 1)],
                    )

            ones_st = bigp.tile([P, NCH], BF, tag="ones_st")
            nc.gpsimd.memset(ones_st[:], 1.0)
            for h in range(2):
                vv = vaug[h][:].rearrange("p (k e) -> p k e", e=33)
                nc.vector.tensor_copy(out=vv[:, :, 32], in_=ones_st[:])

            bankc = 0  # global rotating-slot cursor

            # ---- qkv builds -------------------------------------------------
            # q/k stage g covers two 512-col panels (2*g, 2*g+1); psum rows
            # 64*s + 32*h + d.  One [128,512] fp8 hi-copy + lo-subtract, then
            # 8 fold DMAs [32,512] into the DoubleRow layouts.
            def qk_stage(w_sb, side, g):
                nonlocal bankc
                rt, c0 = rhalf(bankc)
                bankc += 1
                for s in range(2):
                    it = 2 * g + s
                    for cc in range(2):
                        nc.tensor.matmul(
                            rt[64 * s:64 * (s + 1), c0:c0 + 512],
                            lhsT=w_sb[:, cc, :],
                            rhs=xT[cc][:, 512 * it:512 * (it + 1)],
                            start=(cc == 0), stop=(cc == 1),
                            tile_position=(0, 64 * s),
                            skip_group_check=True,
                        )
                hi = stagep.tile([P, 512], E4, tag="hi")
                lo = stagep.tile([P, 512], E4, tag="lo")
                nc.vector.tensor_copy(out=hi[:], in_=rt[:, c0:c0 + 512])
                nc.gpsimd.tensor_tensor(
                    out=lo[:], in0=rt[:, c0:c0 + 512], in1=hi[:],
                    op=ALU.subtract,
                )
                for s in range(2):
                    it = 2 * g + s
                    for h in range(2):
                        r0 = 64 * s + 32 * h
                        if side is qSide:
                            # rows 0:32 = hi, 32:64 = lo; t duplicated later
                            nc.gpsimd.dma_start(
                                out=side[h][0:32, 0, 512 * it:512 * (it + 1)],
                                in_=hi[r0:r0 + 32, :])
                            nc.gpsimd.dma_start(
                                out=side[h][32:64, 0, 512 * it:512 * (it + 1)],
                                in_=lo[r0:r0 + 32, :])
                        else:
                            # cols t=0 = hi, t=1 = lo; rows duplicated later
                            nc.gpsimd.dma_start(
                                out=side[h][0:32, 0, 512 * it:512 * (it + 1)],
                                in_=hi[r0:r0 + 32, :])
                            nc.gpsimd.dma_start(
                                out=side[h][0:32, 1, 512 * it:512 * (it + 1)],
                                in_=lo[r0:r0 + 32, :])

            def v_round(k0):
                nonlocal bankc
                rt2, c02 = rhalf(bankc)
                bankc += 1
                for k in range(k0, k0 + 4):
                    for cc in range(2):
                        nc.tensor.matmul(
                            rt2[:, c02 + 64 * (k - k0):
                                c02 + 64 * (k - k0) + 64],
                            lhsT=xT[cc][:, P * k:P * (k + 1)],
                            rhs=wv_sb[:, cc, :],
                            start=(cc == 0), stop=(cc == 1),
                        )
                sv2 = rt2[:, c02: c02 + 256].rearrange("p (k d) -> p k d", d=64)
                for h in range(2):
                    vv = vaug[h][:].rearrange("p (k e) -> p k e", e=33)
                    nc.vector.tensor_copy(
                        out=vv[:, k0:k0 + 4, 0:32],
                        in_=sv2[:, :, 32 * h:32 * (h + 1)],
                    )

            # prologue: all of q/k/v for BOTH heads (x DMA pieces feed in
            # column order; stage g needs cols 1024g:1024(g+1))
            for g in range(4):
                qk_stage(wq_sb, qSide, g)
                v_round(8 * g)
                v_round(8 * g + 4)
                qk_stage(wk_sb, kSide, g)
            for h in range(2):
                # duplicate: qSide t=1 <- t=0 ; kSide rows 32:64 <- 0:32
                nc.gpsimd.dma_start(out=qSide[h][:, 1, :], in_=qSide[h][:, 0, :])
                nc.gpsimd.dma_start(out=kSide[h][32:64, :, :],
                                    in_=kSide[h][0:32, :, :])

            # ================= attention chunk stream ======================
            def chunk_meta(c):
                h = c // (ITILES * NCH)
                it = (c // NCH) % ITILES
                j = c % NCH
                return h, it, j

            NC_TOT = 2 * ITILES * NCH
            NQ = NC_TOT // 2
            pattern = _make_pattern(NQ, EXP_W)

            slot_of = {}

            def emit_sim(c):
                nonlocal bankc
                h, it, j = chunk_meta(c)
                slot_of[c] = bankc
                rt_, c0 = rhalf(bankc)
                bankc += 1
                nc.tensor.matmul(
                    rt_[:, c0:c0 + 512],
                    lhsT=kSide[h][:, :, P * j:P * (j + 1)],
                    rhs=qSide[h][:, :, 512 * it:512 * (it + 1)],
                    start=True, stop=True,
                    perf_mode=DR,
                )

            def emit_exp(c0):
                # quantum = chunks (c0, c0+1) -> one rot tile, one eslab
                s0 = slot_of[c0]
                assert s0 % 2 == 0 and slot_of[c0 + 1] == s0 + 1, (c0, s0)
                rt_ = rots[(s0 % ROT) // 2]
                es = eslabs[(c0 % EROT) // 2]
                eng = pattern[c0 // 2 % NQ]
                if eng == "A":
                    nc.scalar.activation(es[:], rt_[:], AF.Exp, scale=SCALE)
                elif eng == "V":
                    nc.vector.tensor_scalar(
                        out=es[:].bitcast(I16), in0=rt_[:],
                        scalar1=SCH_A, scalar2=SCH_B,
                        op0=ALU.mult, op1=ALU.add,
                    )
                else:
                    nc.gpsimd.tensor_scalar(
                        out=es[:].bitcast(I16), in0=rt_[:],
                        scalar1=SCH_A, scalar2=SCH_B,
                        op0=ALU.mult, op1=ALU.add,
                    )

            def emit_av(c):
                h, it, j = chunk_meta(c)
                es = eslabs[(c % EROT) // 2]
                e0 = 512 * (c % 2)
                for ic in range(4):
                    nc.tensor.matmul(
                        avp[:, 33 * ic:33 * ic + 33],
                        lhsT=es[:, e0 + 128 * ic:e0 + 128 * (ic + 1)],
                        rhs=vaug[h][:, 33 * j:33 * j + 33],
                        start=(j == 0 and ic == 0), stop=(j == NCH - 1),
                        skip_group_check=True,
                    )

            def emit_itile_stage(h, it):
                dv = avp[:, 0:132].rearrange("p (ic e) -> p ic e", e=33)[:, :, 32]
                r0 = 32 * h + 4 * it
                nc.vector.reciprocal(out=rden[:, r0:r0 + 4], in_=dv)
                for ic in range(4):
                    eng = nc.gpsimd if ic % 2 == 0 else nc.vector
                    eng.tensor_scalar_mul(
                        av_sc[:, 32 * ic:32 * (ic + 1)],
                        avp[:, 33 * ic:33 * ic + 32],
                        rden[:, r0 + ic:r0 + ic + 1],
                    )

            def emit_itile_transpose(h, it):
                # borrows a FULL rot pair (2 slots) so sim-chunk quanta keep
                # their even/odd slot pairing for the full-tile exp reads.
                nonlocal bankc
                assert bankc % 2 == 0, bankc
                rt_, c0 = rhalf(bankc)
                bankc += 2
                for ic in range(4):
                    nc.tensor.transpose(
                        rt_[0:32, c0 + 64 * ic:c0 + 64 * (ic + 1)].bitcast(BF),
                        av_sc[:, 32 * ic:32 * (ic + 1)],
                        identb[:],
                    )
                nc.vector.tensor_copy(
                    out=outT[h][:, 512 * it:512 * (it + 1)],
                    in_=rt_[0:32, c0:c0 + 256].bitcast(BF),
                )

            def emit_y(it):
                for ic in range(4):
                    k = 4 * it + ic
                    cols = slice(256 * (ic % 2), 256 * (ic % 2) + C)
                    for h in range(2):
                        nc.tensor.matmul(
                            tb[:, cols],
                            lhsT=outT[h][:, P * k:P * (k + 1)],
                            rhs=wo_sb[h][:],
                            start=(h == 0), stop=(h == 1),
                            tile_position=(0, 0),
                        )
                    yo = ytmpp.tile([P, C], FP, tag="yo")
                    nc.scalar.copy(out=yo[:], in_=tb[:, cols])
                    nc.sync.dma_start(out=y_d[P * k:P * (k + 1), :], in_=yo[:])

            hooks = {}

            def add_hook(c, fn):
                hooks.setdefault(min(c, NC_TOT - 1), []).append(fn)

            for c in range(NC_TOT):
                emit_sim(c)
                if c >= AV_LAG:
                    # av of c-AV_LAG MUST precede the exp quantum closing at c:
                    # that exp overwrites the eslab cols av(c-AV_LAG) reads.
                    emit_av(c - AV_LAG)
                    ch, cit, cj = chunk_meta(c - AV_LAG)
                    if cj == NCH - 1:
                        emit_itile_stage(ch, cit)
                        add_hook(c + 2, lambda ch=ch, cit=cit:
                                 emit_itile_transpose(ch, cit))
                        if ch == 1 and cit > 0:
                            add_hook(c + 4, lambda cit=cit: emit_y(cit - 1))
                if c % 2 == 1:
                    emit_exp(c - 1)
                for fn in hooks.pop(c, ()):
                    fn()

            # tail: remaining avs, last i-tile stage/transpose, last y projs
            for c in range(NC_TOT - AV_LAG, NC_TOT):
                emit_av(c)
                ch, cit, cj = chunk_meta(c)
                if cj == NCH - 1:
                    emit_itile_stage(ch, cit)
                    emit_itile_transpose(ch, cit)
            for fn_list in [hooks[k] for k in sorted(hooks)]:
                for fn in fn_list:
                    fn()
            emit_y(ITILES - 2)
            emit_y(ITILES - 1)

    _split_excess_waits(nc, mybir)
    return nc


def _split_excess_waits(nc, mybir, maxw=1, carrier_cap=1):
    """Hoist excess semaphore waits onto InstEventSemaphore carriers."""
    skip = {
        "InstEventSemaphore", "InstCall",
        "InstUnconditionalBranch", "InstISA", "InstRegisterMove",
    }
    for f in nc.m.functions:
        for blk in f.blocks:
            idx = 0
            while idx < len(blk.instructions):
                ins = blk.instructions[idx]
                si = getattr(ins, "sync_info", None)
                if (
                    si is not None and si.on_wait and len(si.on_wait) > maxw
                    and type(ins).__name__ not in skip
                ):
                    waits = list(si.on_wait)
                    keep, excess = waits[:maxw], waits[maxw:]
                    at = idx
                    if (at > 0 and type(blk.instructions[at - 1]).__name__
                            == "InstLdweights"):
                        at -= 1
                    n_ins = 0
                    for i in range(0, len(excess), carrier_cap):
                        ev = mybir.InstEventSemaphore(
                            name=nc.get_next_instruction_name(),
                            engine=ins.engine,
                            ins=[], outs=[],
                            sync_info=mybir.SyncInfo(
                                on_wait=excess[i:i + carrier_cap], on_update=[]
                            ),
                        )
                        nc.register_instruction(ev)
                        blk.instructions.insert(at + n_ins, ev)
                        n_ins += 1
                    ins.sync_info = mybir.SyncInfo(
                        on_wait=keep, on_update=list(si.on_update or [])
                    )
                    idx += n_ins
                idx += 1
    return nc


def get_nc():
    if "nc" not in _CACHED:
        _CACHED["nc"] = _build_nc()
    return _CACHED["nc"]


def make_in_maps(x, w_qkv, w_out):
    """Host-side sharding: core c -> batch c//2, heads (c%2)*2, (c%2)*2+1."""
    import ml_dtypes
    B = x.shape[0]
    xf = np.ascontiguousarray(x.reshape(B, N, C))
    in_maps = []
    for core in range(8):
        b, hp = core // 2, core % 2
        h0, h1 = 2 * hp, 2 * hp + 1
        wq = np.concatenate(
            [w_qkv[:, h * DH:(h + 1) * DH] for h in (h0, h1)], axis=1
        )
        wk = np.concatenate(
            [w_qkv[:, 128 + h * DH: 128 + (h + 1) * DH] for h in (h0, h1)], axis=1
        )
        wv = np.concatenate(
            [w_qkv[:, 256 + h * DH: 256 + (h + 1) * DH] for h in (h0, h1)], axis=1
        )
        wo = np.concatenate(
            [w_out[h * DH:(h + 1) * DH, :] for h in (h0, h1)], axis=0
        )
        in_maps.append({
            "xt": np.ascontiguousarray(xf[b].T.astype(ml_dtypes.bfloat16)).view(np.uint16),
            "wq": np.ascontiguousarray(wq.astype(np.float32)),
            "wk": np.ascontiguousarray(wk.astype(np.float32)),
            "wv": np.ascontiguousarray(wv.astype(np.float32)),
            "wo": np.ascontiguousarray(wo.astype(np.float32)),
        })
    return in_maps


def kernel(x, w_qkv, w_out, b_out):
    from concourse.bass_utils import run_bass_kernel_spmd

    nc = get_nc()
    in_maps = make_in_maps(
        np.asarray(x, dtype=np.float32),
        np.asarray(w_qkv, dtype=np.float32),
        np.asarray(w_out, dtype=np.float32),
    )
    res = run_bass_kernel_spmd(nc, in_maps, list(range(8))).results
    B, H, W = 4, 64, 64
    y = np.empty((B, N, C), dtype=np.float32)
    for b in range(B):
        y[b] = res[2 * b]["y"] + res[2 * b + 1]["y"]
    y += np.asarray(b_out, dtype=np.float32)
    return y.reshape(B, H, W, C)


# revision 9
# speedup vs baseline: 1.0836x; 1.0418x over previous
"""Trainium2 Bass kernel v3 for spatial self-attention (nn_Attention_90615220011343).

Per-core (core c -> batch c//2, heads 2*(c%2), 2*(c%2)+1):
    qkv = x @ w_qkv; per head sim^T[j,i] = k^T q; attn = softmax; out = attn@v
    y_partial = sum_h (out_h/den) @ wo_h ; host sums head-pairs + bias.

v3 changes vs v2 (cost-model driven):
  - sim matmul in fp8e4m3 with MatmulPerfMode.DoubleRow: 0.5 cycles/row
    (vs 1.0 for bf16/fp32r) -> 256 PE cycles per [128,512] chunk.
    Precision recovered by error compensation: q = qhi + qlo, k = khi + klo
    (hi = fp8(x), lo = fp8(x - hi)); the 128 DoubleRow contraction slots
    (64 partitions x 2) hold all four cross products (qhi+qlo)x(khi+klo),
    so the product is exact up to the ~0.1% lo-rounding. attn scale (1/sqrt(32))
    is folded into the exp input scaling, not into q.
  - exp runs on THREE engines: ACT (exact, activation Exp with scale),
    DVE + Pool (Schraudolph int16 bitcast = bf16 exp approx). Pattern
    weighted by engine rates (ACT 0.83, DVE 1.04, Pool 1.39 ns/row).
  - q^T/k^T builds write 2-itile/panel stacks into one [128,512] psum
    ([64*s + 32*h + d] partitions), so the fp8 hi-copy + lo-subtract are
    [128,512] ops (4x fewer engine rows); SBUF->SBUF DMAs (cheap issue from
    the gpsimd ring) fold the stacks into the DoubleRow operand layouts:
      qSide[h]: [64, 2, N] rows = (qhi d | qlo d), t duplicated
      kSide[h]: [64, 2, N] cols t = (khi | klo), rows duplicated
  - transposes of the normalized attention output are bf16 (1 c/r, vs fp32r
    4 c/r when free<256); outT copy reads bf16 psum (DVE 2x_1p mode).
"""

import numpy as np

HEADS = 4
DH = 32
N = 4096
C = 256
P = 128
NCH = 32          # j-chunks of 128 tokens
ITILES = 8        # i tiles of 512
ROT = 6           # rotating psum banks for sim chunks
EROT = 12         # eslab rotation depth (chunks)
AV_LAG = 10       # chunks between sim emission and its av matmuls
SCALE = float(DH ** -0.5)
# bf16 Schraudolph exp: round(s*a+b) as int16 IS bf16(exp(s)) up to ~3%
# sawtooth; softmax normalization cancels most. a absorbs the attn scale.
SCH_A = float(2 ** 7 / np.log(2)) * SCALE
SCH_B = float(127 * 2 ** 7) - 7.6
# exp engine pattern per 32 quanta (1 quantum = 2 chunks = [128,1024]):
# weighted by engine throughput ACT:DVE:Pool ~ 13:11:8
EXP_W = {"A": 13, "V": 11, "P": 8}

_CACHED = {}


def _make_pattern(total, weights):
    acc = {k: 0.0 for k in weights}
    wsum = float(sum(weights.values()))
    out = []
    for _ in range(total):
        for k in acc:
            acc[k] += weights[k]
        kbest = max(acc, key=lambda kk: (acc[kk], kk))
        acc[kbest] -= wsum
        out.append(kbest)
    return out


def _build_nc():
    import concourse.bass as bass
    import concourse.mybir as mybir
    from concourse.tile import TileContext
    from concourse.masks import make_identity

    FP = mybir.dt.float32
    BF = mybir.dt.bfloat16
    E4 = mybir.dt.float8e4
    U16 = mybir.dt.uint16
    I16 = mybir.dt.int16
    AF = mybir.ActivationFunctionType
    ALU = mybir.AluOpType
    DR = mybir.MatmulPerfMode.DoubleRow

    nc = bass.Bass(target_bir_lowering=False)
    xt_d = nc.declare_dram_parameter("xt", [C, N], U16, isOutput=False)
    wq_d = nc.declare_dram_parameter("wq", [C, 64], FP, isOutput=False)
    wk_d = nc.declare_dram_parameter("wk", [C, 64], FP, isOutput=False)
    wv_d = nc.declare_dram_parameter("wv", [C, 64], FP, isOutput=False)
    wo_d = nc.declare_dram_parameter("wo", [64, C], FP, isOutput=False)
    y_d = nc.declare_dram_parameter("y", [N, C], FP, isOutput=True)

    with TileContext(nc) as tc:
        with (
            tc.tile_pool(name="const", bufs=1) as constp,
            tc.tile_pool(name="big", bufs=1) as bigp,
            tc.tile_pool(name="ytmp", bufs=4) as ytmpp,
            tc.tile_pool(name="psR", bufs=1, space="PSUM") as psR,
            tc.tile_pool(name="psV", bufs=1, space="PSUM") as psV,
            tc.tile_pool(name="psT", bufs=1, space="PSUM") as psT,
        ):
            ident = constp.tile([P, P], FP, tag="ident")
            make_identity(nc, ident[:])
            identb = constp.tile([P, P], BF, tag="identb")
            nc.vector.tensor_copy(out=identb[:], in_=ident[:])

            # ---- persistent SBUF ----
            xT = [bigp.tile([P, N], BF, tag=f"xT{cc}", name=f"xT{cc}") for cc in range(2)]
            qSide = [bigp.tile([64, 2, N], E4, tag=f"qS{h}", name=f"qS{h}")
                     for h in range(2)]
            kSide = [bigp.tile([64, 2, N], E4, tag=f"kS{h}", name=f"kS{h}")
                     for h in range(2)]
            vaug = [bigp.tile([P, 33 * NCH], BF, tag=f"vaug{h}", name=f"vaug{h}")
                    for h in range(2)]
            outT = bigp.tile([64, N], BF, tag="outT")
            rden = bigp.tile([P, 64], FP, tag="rden")
            av_sc = bigp.tile([P, P], BF, tag="av_sc")
            eslabs = [bigp.tile([P, 1024], BF, tag=f"esl{t}", name=f"esl{t}")
                      for t in range(EROT // 2)]

            wq_sb = bigp.tile([P, 2, 64], BF, tag="wq")
            wk_sb = bigp.tile([P, 2, 64], BF, tag="wk")
            wv_sb = bigp.tile([P, 2, 64], BF, tag="wv")
            wo_sb = bigp.tile([64, C], BF, tag="wo")

            # ---- psum ----
            rots = [psR.tile([P, 1024], FP, tag=f"R{t}", name=f"rotT{t}")
                    for t in range(ROT // 2)]
            avp = psV.tile([P, 512], FP, tag="V")      # cols 0:132 in use
            tb = psT.tile([P, 512], FP, tag="T")       # y projections

            def rhalf(bc):
                return rots[(bc % ROT) // 2], 512 * (bc % 2)

            # ---- weight loads + conversion ----
            wq_st = bigp.tile([P, 2, 64], FP, tag="wq_st")
            wk_st = bigp.tile([P, 2, 64], FP, tag="wk_st")
            wv_st = bigp.tile([P, 2, 64], FP, tag="wv_st")
            wo_st = bigp.tile([64, C], FP, tag="wo_st")
            for cc in range(2):
                nc.sync.dma_start(out=wq_st[:, cc, :], in_=wq_d[cc * P:(cc + 1) * P, :])
                nc.sync.dma_start(out=wk_st[:, cc, :], in_=wk_d[cc * P:(cc + 1) * P, :])
                nc.sync.dma_start(out=wv_st[:, cc, :], in_=wv_d[cc * P:(cc + 1) * P, :])
            nc.sync.dma_start(out=wo_st[:], in_=wo_d[:])
            nc.vector.tensor_copy(out=wq_sb[:], in_=wq_st[:])
            nc.vector.tensor_copy(out=wk_sb[:], in_=wk_st[:])
            nc.vector.tensor_copy(out=wv_sb[:], in_=wv_st[:])
            nc.vector.tensor_copy(out=wo_sb[:], in_=wo_st[:])

            # ---- x load (pre-transposed bf16 from host), 3 DMA rings ----
            dma_engines = [nc.sync, nc.scalar, nc.gpsimd]
            for s in range(4):
                for cc in range(2):
                    dma_engines[(2 * s + cc) % 3].dma_start(
                        out=xT[cc][:, 1024 * s:1024 * (s + 1)].bitcast(U16),
                        in_=xt_d[P * cc:P * (cc + 1),
                                 1024 * s:1024 * (s + 1)],
                    )

            ones_st = bigp.tile([P, NCH], BF, tag="ones_st")
            nc.gpsimd.memset(ones_st[:], 1.0)
            for h in range(2):
                vv = vaug[h][:].rearrange("p (k e) -> p k e", e=33)
                nc.vector.tensor_copy(out=vv[:, :, 32], in_=ones_st[:])

            bankc = 0  # global rotating-slot cursor

            # ---- qkv builds -------------------------------------------------
            # q/k stage g covers FOUR itiles (4g..4g+4) stacked in one
            # [128,1024] psum pair: rows 64*s + 32*h + d, cols 512*c2 hold
            # itile 4g+2s+c2.  Then per (s,h): one [32,1024] fp8 hi-copy and
            # one [32,1024] lo-subtract straight into the DoubleRow operand
            # tiles (partition-shifted engine ops; no staging, no fold DMAs):
            #   qSide[h]: rows 0:32 = hi, 32:64 = lo; t dim duplicated by DMA
            #   kSide[h]: t=0 = hi, t=1 = lo; rows 32:64 duplicated by DMA
            cpeng = [nc.scalar, nc.vector, nc.scalar, nc.gpsimd]
            sbeng = [nc.vector, nc.gpsimd, nc.vector, nc.vector]

            def qk_stage(w_sb, side, g):
                nonlocal bankc
                assert bankc % 2 == 0, bankc
                rt, _ = rhalf(bankc)
                bankc += 2
                for s in range(2):
                    for c2 in range(2):
                        it = 4 * g + 2 * s + c2
                        for cc in range(2):
                            nc.tensor.matmul(
                                rt[64 * s:64 * (s + 1), 512 * c2:512 * (c2 + 1)],
                                lhsT=w_sb[:, cc, :],
                                rhs=xT[cc][:, 512 * it:512 * (it + 1)],
                                start=(cc == 0), stop=(cc == 1),
                                tile_position=(0, 64 * s),
                                skip_group_check=True,
                            )
                for s in range(2):
                    cols = slice(1024 * (2 * g + s), 1024 * (2 * g + s) + 1024)
                    for h in range(2):
                        r0 = 64 * s + 32 * h
                        if side is qSide:
                            hi_ap = side[h][0:32, 0, cols]
                            lo_ap = side[h][32:64, 0, cols]
                        else:
                            hi_ap = side[h][0:32, 0, cols]
                            lo_ap = side[h][0:32, 1, cols]
                        eng = cpeng[2 * s + h]
                        if eng is nc.scalar:
                            eng.copy(out=hi_ap, in_=rt[r0:r0 + 32, :])
                        else:
                            eng.tensor_copy(out=hi_ap, in_=rt[r0:r0 + 32, :])
                        sbeng[2 * s + h].tensor_tensor(
                            out=lo_ap, in0=rt[r0:r0 + 32, :], in1=hi_ap,
                            op=ALU.subtract,
                        )

            def dup_group(g):
                # duplicate this 2048-col group's qSide t dim / kSide rows
                cols = slice(2048 * g, 2048 * (g + 1))
                for h in range(2):
                    nc.sync.dma_start(out=qSide[h][:, 1, cols],
                                      in_=qSide[h][:, 0, cols])
                    nc.sync.dma_start(out=kSide[h][32:64, :, cols],
                                      in_=kSide[h][0:32, :, cols])

            def v_round(k0):
                nonlocal bankc
                rt2, c02 = rhalf(bankc)
                bankc += 1
                for k in range(k0, k0 + 4):
                    for cc in range(2):
                        nc.tensor.matmul(
                            rt2[:, c02 + 64 * (k - k0):
                                c02 + 64 * (k - k0) + 64],
                            lhsT=xT[cc][:, P * k:P * (k + 1)],
                            rhs=wv_sb[:, cc, :],
                            start=(cc == 0), stop=(cc == 1),
                        )
                sv2 = rt2[:, c02: c02 + 256].rearrange("p (k d) -> p k d", d=64)
                for h in range(2):
                    vv = vaug[h][:].rearrange("p (k e) -> p k e", e=33)
                    nc.vector.tensor_copy(
                        out=vv[:, k0:k0 + 4, 0:32],
                        in_=sv2[:, :, 32 * h:32 * (h + 1)],
                    )

            # prologue: all of q/k/v for BOTH heads (x DMA pieces feed in
            # column order; stage g needs cols 2048g:2048(g+1))
            for g in range(2):
                qk_stage(wk_sb, kSide, g)
                v_round(16 * g)
                v_round(16 * g + 4)
                qk_stage(wq_sb, qSide, g)
                v_round(16 * g + 8)
                v_round(16 * g + 12)
                dup_group(g)

            # ================= attention chunk stream ======================
            def chunk_meta(c):
                h = c // (ITILES * NCH)
                it = (c // NCH) % ITILES
                j = c % NCH
                return h, it, j

            NC_TOT = 2 * ITILES * NCH
            NQ = NC_TOT // 2
            pattern = _make_pattern(NQ, EXP_W)

            slot_of = {}

            def emit_sim(c):
                nonlocal bankc
                h, it, j = chunk_meta(c)
                slot_of[c] = bankc
                rt_, c0 = rhalf(bankc)
                bankc += 1
                nc.tensor.matmul(
                    rt_[:, c0:c0 + 512],
                    lhsT=kSide[h][:, :, P * j:P * (j + 1)],
                    rhs=qSide[h][:, :, 512 * it:512 * (it + 1)],
                    start=True, stop=True,
                    perf_mode=DR,
                )

            def emit_exp(c0):
                # quantum = chunks (c0, c0+1) -> one rot tile, one eslab
                s0 = slot_of[c0]
                assert s0 % 2 == 0 and slot_of[c0 + 1] == s0 + 1, (c0, s0)
                rt_ = rots[(s0 % ROT) // 2]
                es = eslabs[(c0 % EROT) // 2]
                eng = pattern[c0 // 2 % NQ]
                if eng == "A":
                    nc.scalar.activation(es[:], rt_[:], AF.Exp, scale=SCALE)
                elif eng == "V":
                    nc.vector.tensor_scalar(
                        out=es[:].bitcast(I16), in0=rt_[:],
                        scalar1=SCH_A, scalar2=SCH_B,
                        op0=ALU.mult, op1=ALU.add,
                    )
                else:
                    nc.gpsimd.tensor_scalar(
                        out=es[:].bitcast(I16), in0=rt_[:],
                        scalar1=SCH_A, scalar2=SCH_B,
                        op0=ALU.mult, op1=ALU.add,
                    )

            def emit_av(c):
                h, it, j = chunk_meta(c)
                es = eslabs[(c % EROT) // 2]
                e0 = 512 * (c % 2)
                for ic in range(4):
                    nc.tensor.matmul(
                        avp[:, 33 * ic:33 * ic + 33],
                        lhsT=es[:, e0 + 128 * ic:e0 + 128 * (ic + 1)],
                        rhs=vaug[h][:, 33 * j:33 * j + 33],
                        start=(j == 0 and ic == 0), stop=(j == NCH - 1),
                        skip_group_check=True,
                    )

            def emit_itile_stage(h, it):
                dv = avp[:, 0:132].rearrange("p (ic e) -> p ic e", e=33)[:, :, 32]
                r0 = 32 * h + 4 * it
                nc.vector.reciprocal(out=rden[:, r0:r0 + 4], in_=dv)
                for ic in range(4):
                    eng = nc.gpsimd if ic % 2 == 0 else nc.vector
                    eng.tensor_scalar_mul(
                        av_sc[:, 32 * ic:32 * (ic + 1)],
                        avp[:, 33 * ic:33 * ic + 32],
                        rden[:, r0 + ic:r0 + ic + 1],
                    )

            def emit_itile_transpose(h, it):
                # borrows a FULL rot pair (2 slots) so sim-chunk quanta keep
                # their even/odd slot pairing for the full-tile exp reads.
                nonlocal bankc
                assert bankc % 2 == 0, bankc
                rt_, c0 = rhalf(bankc)
                bankc += 2
                for ic in range(4):
                    nc.tensor.transpose(
                        rt_[0:32, c0 + 64 * ic:c0 + 64 * (ic + 1)].bitcast(BF),
                        av_sc[:, 32 * ic:32 * (ic + 1)],
                        identb[:],
                    )
                nc.vector.tensor_copy(
                    out=outT[32 * h:32 * (h + 1), 512 * it:512 * (it + 1)],
                    in_=rt_[0:32, c0:c0 + 256].bitcast(BF),
                )

            def emit_y(it):
                for ic in range(4):
                    k = 4 * it + ic
                    cols = slice(256 * (ic % 2), 256 * (ic % 2) + C)
                    nc.tensor.matmul(
                        tb[:, cols],
                        lhsT=outT[:, P * k:P * (k + 1)],
                        rhs=wo_sb[:],
                        start=True, stop=True,
                        tile_position=(0, 0),
                    )
                    yo = ytmpp.tile([P, C], FP, tag="yo")
                    nc.scalar.copy(out=yo[:], in_=tb[:, cols])
                    nc.sync.dma_start(out=y_d[P * k:P * (k + 1), :], in_=yo[:])

            hooks = {}

            def add_hook(c, fn):
                hooks.setdefault(min(c, NC_TOT - 1), []).append(fn)

            for c in range(NC_TOT):
                emit_sim(c)
                if c >= AV_LAG:
                    # av of c-AV_LAG MUST precede the exp quantum closing at c:
                    # that exp overwrites the eslab cols av(c-AV_LAG) reads.
                    emit_av(c - AV_LAG)
                    ch, cit, cj = chunk_meta(c - AV_LAG)
                    if cj == NCH - 1:
                        emit_itile_stage(ch, cit)
                        add_hook(c + 10, lambda ch=ch, cit=cit:
                                 emit_itile_transpose(ch, cit))
                        if ch == 1 and cit > 0:
                            add_hook(c + 14, lambda cit=cit: emit_y(cit - 1))
                if c % 2 == 1:
                    emit_exp(c - 1)
                for fn in hooks.pop(c, ()):
                    fn()

            # tail: remaining avs, last i-tile stage/transpose, last y projs
            for c in range(NC_TOT - AV_LAG, NC_TOT):
                emit_av(c)
                ch, cit, cj = chunk_meta(c)
                if cj == NCH - 1:
                    emit_itile_stage(ch, cit)
                    emit_itile_transpose(ch, cit)
            for fn_list in [hooks[k] for k in sorted(hooks)]:
                for fn in fn_list:
                    fn()
            emit_y(ITILES - 2)
            emit_y(ITILES - 1)

    _split_excess_waits(nc, mybir)
    return nc


def _split_excess_waits(nc, mybir, maxw=1, carrier_cap=1):
    """Hoist excess semaphore waits onto InstEventSemaphore carriers."""
    skip = {
        "InstEventSemaphore", "InstCall",
        "InstUnconditionalBranch", "InstISA", "InstRegisterMove",
    }
    for f in nc.m.functions:
        for blk in f.blocks:
            idx = 0
            while idx < len(blk.instructions):
                ins = blk.instructions[idx]
                si = getattr(ins, "sync_info", None)
                if (
                    si is not None and si.on_wait and len(si.on_wait) > maxw
                    and type(ins).__name__ not in skip
                ):
                    waits = list(si.on_wait)
                    keep, excess = waits[:maxw], waits[maxw:]
                    at = idx
                    if (at > 0 and type(blk.instructions[at - 1]).__name__
                            == "InstLdweights"):
                        at -= 1
                    n_ins = 0
                    for i in range(0, len(excess), carrier_cap):
                        ev = mybir.InstEventSemaphore(
                            name=nc.get_next_instruction_name(),
                            engine=ins.engine,
                            ins=[], outs=[],
                            sync_info=mybir.SyncInfo(
                                on_wait=excess[i:i + carrier_cap], on_update=[]
                            ),
                        )
                        nc.register_instruction(ev)
                        blk.instructions.insert(at + n_ins, ev)
                        n_ins += 1
                    ins.sync_info = mybir.SyncInfo(
                        on_wait=keep, on_update=list(si.on_update or [])
                    )
                    idx += n_ins
                idx += 1
    return nc


def get_nc():
    if "nc" not in _CACHED:
        _CACHED["nc"] = _build_nc()
    return _CACHED["nc"]


def make_in_maps(x, w_qkv, w_out):
    """Host-side sharding: core c -> batch c//2, heads (c%2)*2, (c%2)*2+1."""
    import ml_dtypes
    B = x.shape[0]
    xf = np.ascontiguousarray(x.reshape(B, N, C))
    in_maps = []
    for core in range(8):
        b, hp = core // 2, core % 2
        h0, h1 = 2 * hp, 2 * hp + 1
        wq = np.concatenate(
            [w_qkv[:, h * DH:(h + 1) * DH] for h in (h0, h1)], axis=1
        )
        wk = np.concatenate(
            [w_qkv[:, 128 + h * DH: 128 + (h + 1) * DH] for h in (h0, h1)], axis=1
        )
        wv = np.concatenate(
            [w_qkv[:, 256 + h * DH: 256 + (h + 1) * DH] for h in (h0, h1)], axis=1
        )
        wo = np.concatenate(
            [w_out[h * DH:(h + 1) * DH, :] for h in (h0, h1)], axis=0
        )
        in_maps.append({
            "xt": np.ascontiguousarray(xf[b].T.astype(ml_dtypes.bfloat16)).view(np.uint16),
            "wq": np.ascontiguousarray(wq.astype(np.float32)),
            "wk": np.ascontiguousarray(wk.astype(np.float32)),
            "wv": np.ascontiguousarray(wv.astype(np.float32)),
            "wo": np.ascontiguousarray(wo.astype(np.float32)),
        })
    return in_maps


def kernel(x, w_qkv, w_out, b_out):
    from concourse.bass_utils import run_bass_kernel_spmd

    nc = get_nc()
    in_maps = make_in_maps(
        np.asarray(x, dtype=np.float32),
        np.asarray(w_qkv, dtype=np.float32),
        np.asarray(w_out, dtype=np.float32),
    )
    res = run_bass_kernel_spmd(nc, in_maps, list(range(8))).results
    B, H, W = 4, 64, 64
    y = np.empty((B, N, C), dtype=np.float32)
    for b in range(B):
        y[b] = res[2 * b]["y"] + res[2 * b + 1]["y"]
    y += np.asarray(b_out, dtype=np.float32)
    return y.reshape(B, H, W, C)
